# revision 3
# baseline (speedup 1.0000x reference)
"""Trainium2 Bass kernel for nn_NodeAggregator (gnn message passing / diffpool-style).

Reference math (per batch element b, forward pass only):
    h      = relu(x @ W1 + b1)                      [N, K]
    logits = h @ W2 + b2 + (-1e9)*(1-mask)[:,None]  [N, K]
    S      = softmax(logits, axis=-1)               [N, K]
    pfeat  = S.T @ x                                [K, F]
    pooled = S.T @ adj @ S                          [K, K]
    (threshold/topk/scatter + straight-through estimator is an exact
     no-op in the forward pass: a_sp + (pooled - a_sp) == pooled)
    d      = 1/sqrt(pooled.sum(-1) + 1e-9)
    padj   = pooled * d[:,None] * d[None,:]
    pmask  = ones

Sharding: data-parallel over batch B=8 across the 8 NeuronCores (one batch
element per core); weight matrices replicated. No collectives.

Layout trick: everything stays in natural (row-major) orientation by
re-associating pooled = (adj.T @ S).T @ S, so the adjacency tiles serve
directly as matmul stationary operands and no on-device transpose is ever
needed.  x is additionally passed pre-transposed from the host (xT) for the
h-stage, whose contraction runs over F.

dtypes: the h/logits/softmax/pfeat path runs in fp32 (float32r matmul mode:
full fp32 precision at 1 cycle/row for free-dim >= 256).  The big
adj-contraction (85% of FLOPs and HBM bytes) runs with bf16 inputs and fp32
PSUM accumulation; because adj >= 0 and S >= 0 those sums average out the
rounding noise (~5e-4 relative on padj).
"""

import os
from contextlib import ExitStack

import ml_dtypes
import numpy as np

import concourse.bass as bass
import concourse.tile as tile
from concourse import bacc, mybir
from concourse.bass_utils import run_bass_kernel_spmd

B, N, F, K = 8, 2048, 512, 256
P = 128
NT = N // P   # 16 n-tiles
FT = F // P   # 4 f-tiles
KH = K // P   # 2 k-halves
NCH = 4       # n-chunks for the h-stage (512 wide)
CH = N // NCH
TT_PHASES = int(os.environ.get("NK_TT_PHASES", "4"))
MTP = NT // TT_PHASES  # m-tiles per TT phase
MW = N // TT_PHASES    # m columns per TT phase

F32 = mybir.dt.float32
F32R = mybir.dt.float32r
BF16 = mybir.dt.bfloat16
X = mybir.AxisListType.X
AF = mybir.ActivationFunctionType

# Accuracy/perf knobs. NK_ADJ_F32=1 switches the adj contraction to fp32
# (float32r) end to end: ~1.4x slower (HBM bound) but fully fp32-exact.
ADJ_F32 = os.environ.get("NK_ADJ_F32", "0") == "1"
DT_A = F32 if ADJ_F32 else BF16
NP_A = np.float32 if ADJ_F32 else ml_dtypes.bfloat16


def _mm_dt(ap):
    """Bitcast fp32 matmul operands to float32r (fast fp32 PE mode)."""
    return ap.bitcast(F32R) if ap.dtype == F32 else ap


def _build_kernel(ctx: ExitStack, tc: tile.TileContext, io: dict):
    nc = tc.nc

    consts = ctx.enter_context(tc.tile_pool(name="consts", bufs=1))
    big = ctx.enter_context(tc.tile_pool(name="big", bufs=1))
    apool = ctx.enter_context(tc.tile_pool(name="apool", bufs=6))
    sm = ctx.enter_context(tc.tile_pool(name="sm", bufs=4))
    work = ctx.enter_context(tc.tile_pool(name="work", bufs=3))
    evp = ctx.enter_context(tc.tile_pool(name="evp", bufs=3))
    psum = ctx.enter_context(tc.tile_pool(name="psum", bufs=2, space="PSUM"))
    psum_tt = ctx.enter_context(tc.tile_pool(name="psum_tt", bufs=1, space="PSUM"))

    # ---- constants / weights ----
    w1_sb = consts.tile([P, FT, K], F32R)
    nc.sync.dma_start(w1_sb, io["w1"].rearrange("(ft p) k -> p ft k", p=P))
    w2_sb = consts.tile([P, KH, K], F32R)
    nc.sync.dma_start(w2_sb, io["w2"].rearrange("(h p) k -> p h k", p=P))
    b1_sb = consts.tile([P, KH], F32)
    nc.sync.dma_start(b1_sb, io["b1"].rearrange("(h p) -> p h", p=P))
    b2b_sb = consts.tile([P, K], F32)
    nc.sync.dma_start(b2b_sb, io["b2"].partition_broadcast(P))
    mb_sb = consts.tile([P, NT], F32)
    nc.sync.dma_start(mb_sb, io["mb"].rearrange("(t p) -> p t", p=P))
    eps_sb = consts.tile([1, 1], F32)
    nc.vector.memset(eps_sb, 1e-9)
    ones_a = consts.tile([P, 1], DT_A)
    nc.vector.memset(ones_a, 1.0)
    d_sb = consts.tile([1, K], F32R)

    # ---- big resident tensors ----
    xT_sb = big.tile([P, FT, N], F32R)
    nc.sync.dma_start(xT_sb, io["xT"].rearrange("(ft p) n -> p ft n", p=P))
    x_sb = big.tile([P, NT, F], F32R)
    nc.sync.dma_start(x_sb, io["x"].rearrange("(t p) f -> p t f", p=P))
    hT_sb = big.tile([P, KH, N], F32R)
    S_sb = big.tile([P, NT, K], F32R)
    Sa_sb = big.tile([P, NT, K], DT_A)   # S cast for the adj contraction
    TT_sb = big.tile([P, NT, K], DT_A)   # (adj.T @ S) tiles, m on partitions
    pooled_sb = big.tile([P, KH, K], F32)

    # ---- phase 1: hT[k, n] = relu(W1.T @ xT + b1) ----
    for kh in range(KH):
        for nch in range(NCH):
            ps = psum.tile([P, CH], F32, name="acc")
            for ft in range(FT):
                nc.tensor.matmul(
                    ps,
                    _mm_dt(w1_sb[:, ft, kh * P:(kh + 1) * P]),
                    _mm_dt(xT_sb[:, ft, nch * CH:(nch + 1) * CH]),
                    start=(ft == 0),
                    stop=(ft == FT - 1),
                )
            nc.scalar.activation(
                hT_sb[:, kh, nch * CH:(nch + 1) * CH], ps, AF.Relu,
                bias=b1_sb[:, kh:kh + 1], scale=1.0,
            )

    # ---- phase 2: logits = hT.T @ W2 + b2; S = softmax(logits + maskbias) ----
    for nt in range(NT):
        lp = psum.tile([P, K], F32, name="lg")
        for kh in range(KH):
            nc.tensor.matmul(
                lp,
                _mm_dt(hT_sb[:, kh, nt * P:(nt + 1) * P]),
                _mm_dt(w2_sb[:, kh, :]),
                start=(kh == 0),
                stop=(kh == KH - 1),
            )
        lg = work.tile([P, K], F32, name="lg_sb")
        nc.vector.tensor_add(lg, lp, b2b_sb)
        mx = sm.tile([P, 1], F32, name="mx")
        nc.vector.reduce_max(mx, lg, axis=X)
        eb = sm.tile([P, 1], F32, name="eb")
        nc.vector.tensor_sub(eb, mb_sb[:, nt:nt + 1], mx)  # maskbias - max
        ex = work.tile([P, K], F32, name="ex")
        ssum = sm.tile([P, 1], F32, name="ssum")
        nc.scalar.activation(ex, lg, AF.Exp, bias=eb, scale=1.0, accum_out=ssum)
        rs = sm.tile([P, 1], F32, name="rs")
        nc.vector.reciprocal(rs, ssum)
        nc.vector.tensor_scalar_mul(S_sb[:, nt, :], ex, rs)
        nc.vector.tensor_copy(Sa_sb[:, nt, :], S_sb[:, nt, :])

    # ---- phase 3: pfeat = S.T @ x (fp32 exact) ----
    for kh in range(KH):
        ps = psum.tile([P, F], F32, name="acc")
        for nt in range(NT):
            nc.tensor.matmul(
                ps,
                _mm_dt(S_sb[:, nt, kh * P:(kh + 1) * P]),
                _mm_dt(x_sb[:, nt, :]),
                start=(nt == 0),
                stop=(nt == NT - 1),
            )
        pe = evp.tile([P, F], F32, name="pf_ev")
        nc.vector.tensor_copy(pe, ps)
        nc.sync.dma_start(io["pfeat"][kh * P:(kh + 1) * P, :], pe)

    # ---- phase 4: TT = adj.T @ S, m-split into phases (PSUM budget) ----
    for ph in range(TT_PHASES):
        tts = [psum_tt.tile([P, K], F32, name=f"tt{i}") for i in range(MTP)]
        for nt in range(NT):
            at = apool.tile([P, MW], DT_A, name="aslab")
            nc.sync.dma_start(
                at, io["adj"][nt * P:(nt + 1) * P, ph * MW:(ph + 1) * MW]
            )
            for mi in range(MTP):
                nc.tensor.matmul(
                    tts[mi],
                    _mm_dt(at[:, mi * P:(mi + 1) * P]),
                    _mm_dt(Sa_sb[:, nt, :]),
                    start=(nt == 0),
                    stop=(nt == NT - 1),
                )
        for mi in range(MTP):
            nc.vector.tensor_copy(TT_sb[:, ph * MTP + mi, :], tts[mi])

    # ---- phase 5: pooled = TT.T @ S ----
    pooled_ps = []
    for kh in range(KH):
        pp = psum.tile([P, K], F32, name="lg")
        for mt in range(NT):
            nc.tensor.matmul(
                pp,
                _mm_dt(TT_sb[:, mt, kh * P:(kh + 1) * P]),
                _mm_dt(Sa_sb[:, mt, :]),
                start=(mt == 0),
                stop=(mt == NT - 1),
            )
        pooled_ps.append(pp)
    for kh in range(KH):
        nc.vector.tensor_copy(pooled_sb[:, kh, :], pooled_ps[kh])

    # ---- phase 6: symmetric degree renorm ----
    # row_sum[k] = sum_l pooled[k,l] = sum_m TT[m,k] (S rows sum to 1),
    # computed directly as a row vector so no transpose is needed.
    rst = psum.tile([P, K], F32, name="lg")
    rsp = rst[:1, :]
    for mt in range(NT):
        nc.tensor.matmul(
            rsp, ones_a, _mm_dt(TT_sb[:, mt, :]),
            start=(mt == 0), stop=(mt == NT - 1),
        )
    d_f32 = work.tile([1, K], F32, name="d_f32")
    nc.scalar.activation(d_f32, rsp, AF.Sqrt, bias=eps_sb, scale=1.0)
    nc.vector.reciprocal(d_f32, d_f32)  # d = 1/sqrt(row_sum + eps), [1, K]
    nc.vector.tensor_copy(d_sb, d_f32)

    for kh in range(KH):
        dd = psum.tile([P, K], F32, name="lg")
        # dd[i, j] = d[kh*P+i] * d[j]  (outer product via K=1 matmul)
        nc.tensor.matmul(
            dd, d_sb[:1, kh * P:(kh + 1) * P], d_sb[:1, :], start=True, stop=True
        )
        pa = evp.tile([P, K], F32, name="pa_ev")
        nc.vector.tensor_mul(pa, pooled_sb[:, kh, :], dd)
        nc.sync.dma_start(io["padj"][kh * P:(kh + 1) * P, :], pa)


_CACHE = {}


def _get_nc():
    key = "nc"
    if key in _CACHE:
        return _CACHE[key]
    nc = bacc.Bacc(
        "TRN2", target_bir_lowering=False, debug=False, enable_asserts=True
    )
    io = {
        "xT": nc.dram_tensor("xT", [F, N], F32R, kind="ExternalInput").ap(),
        "x": nc.dram_tensor("x", [N, F], F32R, kind="ExternalInput").ap(),
        "adj": nc.dram_tensor("adj", [N, N], DT_A, kind="ExternalInput").ap(),
        "w1": nc.dram_tensor("w1", [F, K], F32R, kind="ExternalInput").ap(),
        "w2": nc.dram_tensor("w2", [K, K], F32R, kind="ExternalInput").ap(),
        "b1": nc.dram_tensor("b1", [K], F32, kind="ExternalInput").ap(),
        "b2": nc.dram_tensor("b2", [K], F32, kind="ExternalInput").ap(),
        "mb": nc.dram_tensor("mb", [N], F32, kind="ExternalInput").ap(),
        "pfeat": nc.dram_tensor("pfeat", [K, F], F32, kind="ExternalOutput").ap(),
        "padj": nc.dram_tensor("padj", [K, K], F32, kind="ExternalOutput").ap(),
    }
    with tile.TileContext(nc) as tc, ExitStack() as ctx:
        _build_kernel(ctx, tc, io)
    nc.compile()
    _CACHE[key] = nc
    return nc


def make_in_maps(x, adj, mask, W1, b1, W2, b2):
    """Build the per-core input maps from the full (unsharded) inputs."""
    x = np.asarray(x, np.float32)
    adj = np.asarray(adj, np.float32)
    mask = np.asarray(mask, np.float32)
    w1 = np.ascontiguousarray(np.asarray(W1, np.float32))
    w2 = np.ascontiguousarray(np.asarray(W2, np.float32))
    b1v = np.ascontiguousarray(np.asarray(b1, np.float32))
    b2v = np.ascontiguousarray(np.asarray(b2, np.float32))
    in_maps = []
    for b in range(B):
        xb = np.ascontiguousarray(x[b])
        in_maps.append({
            "xT": np.ascontiguousarray(xb.T),
            "x": xb,
            "adj": np.ascontiguousarray(adj[b].astype(NP_A)),
            "w1": w1,
            "w2": w2,
            "b1": b1v,
            "b2": b2v,
            "mb": np.ascontiguousarray((-1e9 * (1.0 - mask[b])).astype(np.float32)),
        })
    return in_maps


def run(x, adj, mask, W1, b1, W2, b2, trace=False):
    nc = _get_nc()
    in_maps = make_in_maps(x, adj, mask, W1, b1, W2, b2)
    res = run_bass_kernel_spmd(nc, in_maps, core_ids=list(range(B)), trace=trace)
    pfeat = np.stack([res.results[b]["pfeat"] for b in range(B)]).astype(np.float32)
    padj = np.stack([res.results[b]["padj"] for b in range(B)]).astype(np.float32)
    pmask = np.ones((B, K), np.float32)
    return (pfeat, padj, pmask), res


def kernel(x, adj, mask, W1, b1, W2, b2):
    out, _ = run(x, adj, mask, W1, b1, W2, b2, trace=False)
    return out


# revision 5
# speedup vs baseline: 1.0565x; 1.0565x over previous
"""Trainium2 Bass kernel for nn_NodeAggregator (gnn message passing / diffpool-style).

Reference math (per batch element b, forward pass only):
    h      = relu(x @ W1 + b1)                      [N, K]
    logits = h @ W2 + b2 + (-1e9)*(1-mask)[:,None]  [N, K]
    S      = softmax(logits, axis=-1)               [N, K]
    pfeat  = S.T @ x                                [K, F]
    pooled = S.T @ adj @ S                          [K, K]
    (threshold/topk/scatter + straight-through estimator is an exact
     no-op in the forward pass: a_sp + (pooled - a_sp) == pooled)
    d      = 1/sqrt(pooled.sum(-1) + 1e-9)
    padj   = pooled * d[:,None] * d[None,:]
    pmask  = ones

Sharding: data-parallel over batch B=8 across the 8 NeuronCores (one batch
element per core); weight matrices replicated. No collectives.

Layout trick: everything stays in natural (row-major) orientation by
re-associating pooled = (adj.T @ S).T @ S, so the adjacency tiles serve
directly as matmul stationary operands and no on-device transpose is ever
needed.  x is additionally passed pre-transposed from the host (xT) for the
h-stage, whose contraction runs over F.

dtypes: the h/logits/softmax/pfeat path runs in fp32 (float32r matmul mode:
full fp32 precision at 1 cycle/row for free-dim >= 256).  The big
adj-contraction (85% of FLOPs and HBM bytes) runs with bf16 inputs and fp32
PSUM accumulation; because adj >= 0 and S >= 0 those sums average out the
rounding noise (~5e-4 relative on padj).
"""

import os
from contextlib import ExitStack

import ml_dtypes
import numpy as np

import concourse.bass as bass
import concourse.tile as tile
from concourse import bacc, mybir
from concourse.bass_utils import run_bass_kernel_spmd

B, N, F, K = 8, 2048, 512, 256
P = 128
NT = N // P   # 16 n-tiles
FT = F // P   # 4 f-tiles
KH = K // P   # 2 k-halves
NCH = 4       # n-chunks for the h-stage (512 wide)
CH = N // NCH
TT_PHASES = int(os.environ.get("NK_TT_PHASES", "4"))
MTP = NT // TT_PHASES  # m-tiles per TT phase
MW = N // TT_PHASES    # m columns per TT phase

F32 = mybir.dt.float32
F32R = mybir.dt.float32r
BF16 = mybir.dt.bfloat16
X = mybir.AxisListType.X
AF = mybir.ActivationFunctionType

# Accuracy/perf knobs. NK_ADJ_F32=1 switches the adj contraction to fp32
# (float32r) end to end: ~1.4x slower (HBM bound) but fully fp32-exact.
ADJ_F32 = os.environ.get("NK_ADJ_F32", "0") == "1"
DT_A = F32 if ADJ_F32 else BF16
NP_A = np.float32 if ADJ_F32 else ml_dtypes.bfloat16


def _mm_dt(ap):
    """Bitcast fp32 matmul operands to float32r (fast fp32 PE mode)."""
    return ap.bitcast(F32R) if ap.dtype == F32 else ap


def _build_kernel(ctx: ExitStack, tc: tile.TileContext, io: dict):
    nc = tc.nc

    consts = ctx.enter_context(tc.tile_pool(name="consts", bufs=1))
    big = ctx.enter_context(tc.tile_pool(name="big", bufs=1))
    apool = ctx.enter_context(tc.tile_pool(name="apool", bufs=6))
    sm = ctx.enter_context(tc.tile_pool(name="sm", bufs=4))
    work = ctx.enter_context(tc.tile_pool(name="work", bufs=3))
    evp = ctx.enter_context(tc.tile_pool(name="evp", bufs=3))
    psum = ctx.enter_context(tc.tile_pool(name="psum", bufs=2, space="PSUM"))
    psum_tt = ctx.enter_context(tc.tile_pool(name="psum_tt", bufs=1, space="PSUM"))

    # ---- constants / weights ----
    w1_sb = consts.tile([P, FT, K], F32R)
    nc.sync.dma_start(w1_sb, io["w1"].rearrange("(ft p) k -> p ft k", p=P))
    w2_sb = consts.tile([P, KH, K], F32R)
    nc.sync.dma_start(w2_sb, io["w2"].rearrange("(h p) k -> p h k", p=P))
    b1_sb = consts.tile([P, KH], F32)
    nc.sync.dma_start(b1_sb, io["b1"].rearrange("(h p) -> p h", p=P))
    b2b_sb = consts.tile([P, K], F32)
    nc.sync.dma_start(b2b_sb, io["b2"].partition_broadcast(P))
    mb_sb = consts.tile([P, NT], F32)
    nc.sync.dma_start(mb_sb, io["mb"].rearrange("(t p) -> p t", p=P))
    eps_sb = consts.tile([1, 1], F32)
    nc.vector.memset(eps_sb, 1e-9)
    ones_a = consts.tile([P, 1], DT_A)
    nc.vector.memset(ones_a, 1.0)
    d_sb = consts.tile([1, K], F32R)

    # ---- big resident tensors ----
    xT_sb = big.tile([P, FT, N], F32R)
    nc.sync.dma_start(xT_sb, io["xT"].rearrange("(ft p) n -> p ft n", p=P))
    x_sb = big.tile([P, NT, F], F32R)
    nc.sync.dma_start(x_sb, io["x"].rearrange("(t p) f -> p t f", p=P))
    hT_sb = big.tile([P, KH, N], F32R)
    S_sb = big.tile([P, NT, K], F32R)
    Sa_sb = big.tile([P, NT, K], DT_A)   # S cast for the adj contraction
    TT_sb = big.tile([P, NT, K], DT_A)   # (adj.T @ S) tiles, m on partitions
    pooled_sb = big.tile([P, KH, K], F32)

    # ---- phase 1: hT[k, n] = relu(W1.T @ xT + b1) ----
    for kh in range(KH):
        for nch in range(NCH):
            ps = psum.tile([P, CH], F32, name="acc")
            for ft in range(FT):
                nc.tensor.matmul(
                    ps,
                    _mm_dt(w1_sb[:, ft, kh * P:(kh + 1) * P]),
                    _mm_dt(xT_sb[:, ft, nch * CH:(nch + 1) * CH]),
                    start=(ft == 0),
                    stop=(ft == FT - 1),
                )
            nc.scalar.activation(
                hT_sb[:, kh, nch * CH:(nch + 1) * CH], ps, AF.Relu,
                bias=b1_sb[:, kh:kh + 1], scale=1.0,
            )

    # ---- phase 2: logits = hT.T @ W2 + b2; S = softmax(logits + maskbias) ----
    for nt in range(NT):
        lp = psum.tile([P, K], F32, name="lg")
        for kh in range(KH):
            nc.tensor.matmul(
                lp,
                _mm_dt(hT_sb[:, kh, nt * P:(nt + 1) * P]),
                _mm_dt(w2_sb[:, kh, :]),
                start=(kh == 0),
                stop=(kh == KH - 1),
            )
        lg = work.tile([P, K], F32, name="lg_sb")
        nc.vector.tensor_add(lg, lp, b2b_sb)
        mx = sm.tile([P, 1], F32, name="mx")
        nc.vector.reduce_max(mx, lg, axis=X)
        eb = sm.tile([P, 1], F32, name="eb")
        nc.vector.tensor_sub(eb, mb_sb[:, nt:nt + 1], mx)  # maskbias - max
        ex = work.tile([P, K], F32, name="ex")
        ssum = sm.tile([P, 1], F32, name="ssum")
        nc.scalar.activation(ex, lg, AF.Exp, bias=eb, scale=1.0, accum_out=ssum)
        rs = sm.tile([P, 1], F32, name="rs")
        nc.vector.reciprocal(rs, ssum)
        nc.vector.tensor_scalar_mul(S_sb[:, nt, :], ex, rs)
        nc.vector.tensor_copy(Sa_sb[:, nt, :], S_sb[:, nt, :])

    # ---- phase 3: pfeat = S.T @ x (fp32 exact) ----
    for kh in range(KH):
        ps = psum.tile([P, F], F32, name="acc")
        for nt in range(NT):
            nc.tensor.matmul(
                ps,
                _mm_dt(S_sb[:, nt, kh * P:(kh + 1) * P]),
                _mm_dt(x_sb[:, nt, :]),
                start=(nt == 0),
                stop=(nt == NT - 1),
            )
        pe = evp.tile([P, F], F32, name="pf_ev")
        nc.vector.tensor_copy(pe, ps)
        nc.sync.dma_start(io["pfeat"][kh * P:(kh + 1) * P, :], pe)

    # ---- phase 4: TT = adj.T @ S, m-split into phases (PSUM budget) ----
    for ph in range(TT_PHASES):
        tts = [psum_tt.tile([P, K], F32, name=f"tt{i}") for i in range(MTP)]
        for nt in range(NT):
            at = apool.tile([P, MW], DT_A, name="aslab")
            nc.sync.dma_start(
                at, io["adj"][nt * P:(nt + 1) * P, ph * MW:(ph + 1) * MW]
            )
            for mi in range(MTP):
                nc.tensor.matmul(
                    tts[mi],
                    _mm_dt(at[:, mi * P:(mi + 1) * P]),
                    _mm_dt(Sa_sb[:, nt, :]),
                    start=(nt == 0),
                    stop=(nt == NT - 1),
                )
        for mi in range(MTP):
            nc.vector.tensor_copy(TT_sb[:, ph * MTP + mi, :], tts[mi])

    # ---- phase 5: pooled = TT.T @ S ----
    pooled_ps = []
    for kh in range(KH):
        pp = psum.tile([P, K], F32, name="lg")
        for mt in range(NT):
            nc.tensor.matmul(
                pp,
                _mm_dt(TT_sb[:, mt, kh * P:(kh + 1) * P]),
                _mm_dt(Sa_sb[:, mt, :]),
                start=(mt == 0),
                stop=(mt == NT - 1),
            )
        pooled_ps.append(pp)
    for kh in range(KH):
        nc.vector.tensor_copy(pooled_sb[:, kh, :], pooled_ps[kh])

    # ---- phase 6: symmetric degree renorm ----
    # row_sum[k] = sum_l pooled[k,l] = sum_m TT[m,k] (S rows sum to 1),
    # computed directly as a row vector so no transpose is needed.
    rst = psum.tile([P, K], F32, name="lg")
    rsp = rst[:1, :]
    for mt in range(NT):
        nc.tensor.matmul(
            rsp, ones_a, _mm_dt(TT_sb[:, mt, :]),
            start=(mt == 0), stop=(mt == NT - 1),
        )
    d_f32 = work.tile([1, K], F32, name="d_f32")
    nc.scalar.activation(d_f32, rsp, AF.Sqrt, bias=eps_sb, scale=1.0)
    nc.vector.reciprocal(d_f32, d_f32)  # d = 1/sqrt(row_sum + eps), [1, K]
    nc.vector.tensor_copy(d_sb, d_f32)

    for kh in range(KH):
        dd = psum.tile([P, K], F32, name="lg")
        # dd[i, j] = d[kh*P+i] * d[j]  (outer product via K=1 matmul)
        nc.tensor.matmul(
            dd, d_sb[:1, kh * P:(kh + 1) * P], d_sb[:1, :], start=True, stop=True
        )
        pa = evp.tile([P, K], F32, name="pa_ev")
        nc.vector.tensor_mul(pa, pooled_sb[:, kh, :], dd)
        nc.sync.dma_start(io["padj"][kh * P:(kh + 1) * P, :], pa)


_CACHE = {}


def _get_nc():
    key = "nc"
    if key in _CACHE:
        return _CACHE[key]
    nc = bacc.Bacc(
        "TRN2", target_bir_lowering=False, debug=False, enable_asserts=True
    )
    io = {
        "xT": nc.dram_tensor("xT", [F, N], F32R, kind="ExternalInput").ap(),
        "x": nc.dram_tensor("x", [N, F], F32R, kind="ExternalInput").ap(),
        "adj": nc.dram_tensor("adj", [N, N], DT_A, kind="ExternalInput").ap(),
        "w1": nc.dram_tensor("w1", [F, K], F32R, kind="ExternalInput").ap(),
        "w2": nc.dram_tensor("w2", [K, K], F32R, kind="ExternalInput").ap(),
        "b1": nc.dram_tensor("b1", [K], F32, kind="ExternalInput").ap(),
        "b2": nc.dram_tensor("b2", [K], F32, kind="ExternalInput").ap(),
        "mb": nc.dram_tensor("mb", [N], F32, kind="ExternalInput").ap(),
        "pfeat": nc.dram_tensor("pfeat", [K, F], F32, kind="ExternalOutput").ap(),
        "padj": nc.dram_tensor("padj", [K, K], F32, kind="ExternalOutput").ap(),
    }
    with tile.TileContext(nc) as tc, ExitStack() as ctx:
        _build_kernel(ctx, tc, io)
    nc.compile()
    _CACHE[key] = nc
    return nc


def make_in_maps(x, adj, mask, W1, b1, W2, b2):
    """Build the per-core input maps from the full (unsharded) inputs."""
    x = np.asarray(x, np.float32)
    adj = np.asarray(adj, np.float32)
    mask = np.asarray(mask, np.float32)
    w1 = np.ascontiguousarray(np.asarray(W1, np.float32))
    w2 = np.ascontiguousarray(np.asarray(W2, np.float32))
    b1v = np.ascontiguousarray(np.asarray(b1, np.float32))
    b2v = np.ascontiguousarray(np.asarray(b2, np.float32))
    in_maps = []
    for b in range(B):
        xb = np.ascontiguousarray(x[b])
        in_maps.append({
            "xT": np.ascontiguousarray(xb.T),
            "x": xb,
            "adj": np.ascontiguousarray(adj[b].astype(NP_A)),
            "w1": w1,
            "w2": w2,
            "b1": b1v,
            "b2": b2v,
            "mb": np.ascontiguousarray((-1e9 * (1.0 - mask[b])).astype(np.float32)),
        })
    return in_maps


def run(x, adj, mask, W1, b1, W2, b2, trace=False):
    nc = _get_nc()
    in_maps = make_in_maps(x, adj, mask, W1, b1, W2, b2)
    res = run_bass_kernel_spmd(nc, in_maps, core_ids=list(range(B)), trace=trace)
    pfeat = np.stack([res.results[b]["pfeat"] for b in range(B)]).astype(np.float32)
    padj = np.stack([res.results[b]["padj"] for b in range(B)]).astype(np.float32)
    pmask = np.ones((B, K), np.float32)
    return (pfeat, padj, pmask), res


def kernel(x, adj, mask, W1, b1, W2, b2):
    out, _ = run(x, adj, mask, W1, b1, W2, b2, trace=False)
    return out


# revision 8
# speedup vs baseline: 1.2148x; 1.1498x over previous
"""Trainium2 Bass kernel for nn_NodeAggregator (gnn message passing / diffpool-style).

Reference math (per batch element b, forward pass only):
    h      = relu(x @ W1 + b1)                      [N, K]
    logits = h @ W2 + b2 + (-1e9)*(1-mask)[:,None]  [N, K]
    S      = softmax(logits, axis=-1)               [N, K]
    pfeat  = S.T @ x                                [K, F]
    pooled = S.T @ adj @ S                          [K, K]
    (threshold/topk/scatter + straight-through estimator is an exact
     no-op in the forward pass: a_sp + (pooled - a_sp) == pooled)
    d      = 1/sqrt(pooled.sum(-1) + 1e-9)
    padj   = pooled * d[:,None] * d[None,:]
    pmask  = ones

Sharding: data-parallel over batch B=8 across the 8 NeuronCores (one batch
element per core); weight matrices replicated. No collectives.

Layout trick: everything stays in natural (row-major) orientation by
re-associating pooled = (adj.T @ S).T @ S, so the adjacency tiles serve
directly as matmul stationary operands and no on-device transpose is ever
needed.  x is additionally passed pre-transposed from the host (xT) for the
h-stage, whose contraction runs over F.

dtypes: the h/logits/softmax/pfeat path runs in fp32 (float32r matmul mode:
full fp32 precision at 1 cycle/row for free-dim >= 256).  The big
adj-contraction (85% of FLOPs and HBM bytes) runs with bf16 inputs and fp32
PSUM accumulation; because adj >= 0 and S >= 0 those sums average out the
rounding noise (~5e-4 relative on padj).
"""

import os
from contextlib import ExitStack

import ml_dtypes
import numpy as np

import concourse.bass as bass
import concourse.tile as tile
from concourse import bacc, mybir
from concourse.masks import make_identity
from concourse.bass_utils import run_bass_kernel_spmd

B, N, F, K = 8, 2048, 512, 256
P = 128
NT = N // P   # 16 n-tiles
FT = F // P   # 4 f-tiles
KH = K // P   # 2 k-halves
NCH = 4       # n-chunks for the h-stage (512 wide)
CH = N // NCH
TT_PHASES = int(os.environ.get("NK_TT_PHASES", "4"))
MTP = NT // TT_PHASES  # m-tiles per TT phase
MW = N // TT_PHASES    # m columns per TT phase

F32 = mybir.dt.float32
F32R = mybir.dt.float32r
BF16 = mybir.dt.bfloat16
X = mybir.AxisListType.X
AF = mybir.ActivationFunctionType

# Accuracy/perf knobs. NK_ADJ_F32=1 switches the adj contraction to fp32
# (float32r) end to end: ~1.4x slower (HBM bound) but fully fp32-exact.
ADJ_F32 = os.environ.get("NK_ADJ_F32", "0") == "1"
DT_A = F32 if ADJ_F32 else BF16
NP_A = np.float32 if ADJ_F32 else ml_dtypes.bfloat16


def _mm_dt(ap):
    """Bitcast fp32 matmul operands to float32r (fast fp32 PE mode)."""
    return ap.bitcast(F32R) if ap.dtype == F32 else ap


def _build_kernel(ctx: ExitStack, tc: tile.TileContext, io: dict):
    nc = tc.nc

    consts = ctx.enter_context(tc.tile_pool(name="consts", bufs=1))
    big = ctx.enter_context(tc.tile_pool(name="big", bufs=1))
    apool = ctx.enter_context(tc.tile_pool(name="apool", bufs=6))
    sm = ctx.enter_context(tc.tile_pool(name="sm", bufs=4))
    work = ctx.enter_context(tc.tile_pool(name="work", bufs=3))
    evp = ctx.enter_context(tc.tile_pool(name="evp", bufs=3))
    psum = ctx.enter_context(tc.tile_pool(name="psum", bufs=2, space="PSUM"))
    psum_tt = ctx.enter_context(tc.tile_pool(name="psum_tt", bufs=1, space="PSUM"))

    # ---- constants / weights ----
    w1_sb = consts.tile([P, FT, K], F32R)
    nc.sync.dma_start(w1_sb, io["w1"].rearrange("(ft p) k -> p ft k", p=P))
    w2_sb = consts.tile([P, KH, K], F32R)
    nc.sync.dma_start(w2_sb, io["w2"].rearrange("(h p) k -> p h k", p=P))
    b1_sb = consts.tile([P, KH], F32)
    nc.sync.dma_start(b1_sb, io["b1"].rearrange("(h p) -> p h", p=P))
    b2b_sb = consts.tile([P, K], F32)
    nc.sync.dma_start(b2b_sb, io["b2"].partition_broadcast(P))
    mb_sb = consts.tile([P, NT], F32)
    nc.sync.dma_start(mb_sb, io["mb"].rearrange("(t p) -> p t", p=P))
    eps_sb = consts.tile([1, 1], F32)
    nc.vector.memset(eps_sb, 1e-9)
    ones_a = consts.tile([P, 1], DT_A)
    nc.vector.memset(ones_a, 1.0)
    d_sb = consts.tile([1, K], F32R)

    # ---- big resident tensors ----
    xT_sb = big.tile([P, FT, N], F32R)
    xT_r = io["xT"].rearrange("(ft p) n -> p ft n", p=P)
    for c in range(NCH):
        nc.sync.dma_start(
            xT_sb[:, :, c * CH:(c + 1) * CH], xT_r[:, :, c * CH:(c + 1) * CH]
        )
    x_sb = big.tile([P, NT, F], F32R)
    x_r = io["x"].rearrange("(t p) f -> p t f", p=P)
    for c in range(2):
        nc.sync.dma_start(
            x_sb[:, c * 8:(c + 1) * 8, :], x_r[:, c * 8:(c + 1) * 8, :]
        )
    hT_sb = big.tile([P, KH, N], F32R)
    S_sb = big.tile([P, NT, K], F32R)
    Sa_sb = big.tile([P, NT, K], DT_A)   # S cast for the adj contraction
    T_sb = big.tile([P, KH, N], DT_A)    # S.T @ adj, k on partitions
    TT_sb = big.tile([P, NT, K], DT_A)   # its transpose, m on partitions
    pooled_sb = big.tile([P, KH, K], F32)
    ident_sb = consts.tile([P, P], DT_A)
    make_identity(nc, ident_sb)

    # ---- phase 1: hT[k, n] = relu(W1.T @ xT + b1) ----
    for kh in range(KH):
        for nch in range(NCH):
            ps = psum.tile([P, CH], F32, name="acc")
            for ft in range(FT):
                nc.tensor.matmul(
                    ps,
                    _mm_dt(w1_sb[:, ft, kh * P:(kh + 1) * P]),
                    _mm_dt(xT_sb[:, ft, nch * CH:(nch + 1) * CH]),
                    start=(ft == 0),
                    stop=(ft == FT - 1),
                )
            nc.scalar.activation(
                hT_sb[:, kh, nch * CH:(nch + 1) * CH], ps, AF.Relu,
                bias=b1_sb[:, kh:kh + 1], scale=1.0,
            )

    # ---- phase 2: logits = hT.T @ W2 + b2; S = softmax(logits + maskbias) ----
    for nt in range(NT):
        lp = psum.tile([P, K], F32, name="lg")
        for kh in range(KH):
            nc.tensor.matmul(
                lp,
                _mm_dt(hT_sb[:, kh, nt * P:(nt + 1) * P]),
                _mm_dt(w2_sb[:, kh, :]),
                start=(kh == 0),
                stop=(kh == KH - 1),
            )
        lg = work.tile([P, K], F32, name="lg_sb")
        nc.vector.tensor_add(lg, lp, b2b_sb)
        mx = sm.tile([P, 1], F32, name="mx")
        nc.vector.reduce_max(mx, lg, axis=X)
        eb = sm.tile([P, 1], F32, name="eb")
        nc.vector.tensor_sub(eb, mb_sb[:, nt:nt + 1], mx)  # maskbias - max
        ex = work.tile([P, K], F32, name="ex")
        ssum = sm.tile([P, 1], F32, name="ssum")
        nc.scalar.activation(ex, lg, AF.Exp, bias=eb, scale=1.0, accum_out=ssum)
        rs = sm.tile([P, 1], F32, name="rs")
        nc.vector.reciprocal(rs, ssum)
        nc.vector.tensor_scalar_mul(S_sb[:, nt, :], ex, rs)
        nc.vector.tensor_copy(Sa_sb[:, nt, :], S_sb[:, nt, :])

    # ---- phase 3: pfeat = S.T @ x (fp32 exact) ----
    for kh in range(KH):
        ps = psum.tile([P, F], F32, name="acc")
        for nt in range(NT):
            nc.tensor.matmul(
                ps,
                _mm_dt(S_sb[:, nt, kh * P:(kh + 1) * P]),
                _mm_dt(x_sb[:, nt, :]),
                start=(nt == 0),
                stop=(nt == NT - 1),
            )
        pe = evp.tile([P, F], F32, name="pf_ev")
        nc.vector.tensor_copy(pe, ps)
        nc.sync.dma_start(io["pfeat"][kh * P:(kh + 1) * P, :], pe)

    # ---- phase 4: T[k, m] = S.T @ adj with S stationary (few, reused
    # LDWEIGHTS) and adj as the wide moving operand; adj is streamed in
    # 1024-column half-slabs.  T is evicted as bf16 and PE-transposed
    # tile-by-tile into TT[m, k] for the pooled contraction. ----
    MH = 2
    MHW = N // MH  # 1024 adj columns per half
    MC = MHW // 512  # 512-wide matmul chunks per half
    for mh in range(MH):
        tacc = {
            (kh, mc): psum_tt.tile([P, 512], F32, name=f"T{kh}{mc}")
            for kh in range(KH)
            for mc in range(MC)
        }
        for nt in range(NT):
            at = apool.tile([P, MHW], DT_A, name="aslab")
            nc.sync.dma_start(
                at, io["adj"][nt * P:(nt + 1) * P, mh * MHW:(mh + 1) * MHW]
            )
            for kh in range(KH):
                for mc in range(MC):
                    nc.tensor.matmul(
                        tacc[(kh, mc)],
                        _mm_dt(Sa_sb[:, nt, kh * P:(kh + 1) * P]),
                        _mm_dt(at[:, mc * 512:(mc + 1) * 512]),
                        start=(nt == 0),
                        stop=(nt == NT - 1),
                    )
        for kh in range(KH):
            for mc in range(MC):
                nc.vector.tensor_copy(
                    T_sb[:, kh, mh * MHW + mc * 512:mh * MHW + (mc + 1) * 512],
                    tacc[(kh, mc)],
                )
    # transpose T -> TT (PE transpose, bf16, one [128,128] block at a time)
    for mt in range(NT):
        for kh in range(KH):
            tp = psum.tile([P, K], DT_A, name="lg")
            nc.tensor.transpose(
                tp[:, :P], T_sb[:, kh, mt * P:(mt + 1) * P], ident_sb
            )
            nc.vector.tensor_copy(TT_sb[:, mt, kh * P:(kh + 1) * P], tp[:, :P])

    # ---- phase 5: pooled = TT.T @ S ----
    pooled_ps = []
    for kh in range(KH):
        pp = psum.tile([P, K], F32, name="lg")
        for mt in range(NT):
            nc.tensor.matmul(
                pp,
                _mm_dt(TT_sb[:, mt, kh * P:(kh + 1) * P]),
                _mm_dt(Sa_sb[:, mt, :]),
                start=(mt == 0),
                stop=(mt == NT - 1),
            )
        pooled_ps.append(pp)
    for kh in range(KH):
        nc.vector.tensor_copy(pooled_sb[:, kh, :], pooled_ps[kh])

    # ---- phase 6: symmetric degree renorm ----
    # row_sum[k] = sum_l pooled[k,l] = sum_m TT[m,k] (S rows sum to 1),
    # computed directly as a row vector so no transpose is needed.
    rst = psum.tile([P, K], F32, name="lg")
    rsp = rst[:1, :]
    for mt in range(NT):
        nc.tensor.matmul(
            rsp, ones_a, _mm_dt(TT_sb[:, mt, :]),
            start=(mt == 0), stop=(mt == NT - 1),
        )
    d_f32 = work.tile([1, K], F32, name="d_f32")
    nc.scalar.activation(d_f32, rsp, AF.Sqrt, bias=eps_sb, scale=1.0)
    nc.vector.reciprocal(d_f32, d_f32)  # d = 1/sqrt(row_sum + eps), [1, K]
    nc.vector.tensor_copy(d_sb, d_f32)

    for kh in range(KH):
        dd = psum.tile([P, K], F32, name="lg")
        # dd[i, j] = d[kh*P+i] * d[j]  (outer product via K=1 matmul)
        nc.tensor.matmul(
            dd, d_sb[:1, kh * P:(kh + 1) * P], d_sb[:1, :], start=True, stop=True
        )
        pa = evp.tile([P, K], F32, name="pa_ev")
        nc.vector.tensor_mul(pa, pooled_sb[:, kh, :], dd)
        nc.sync.dma_start(io["padj"][kh * P:(kh + 1) * P, :], pa)


_CACHE = {}


def _get_nc():
    key = "nc"
    if key in _CACHE:
        return _CACHE[key]
    nc = bacc.Bacc(
        "TRN2", target_bir_lowering=False, debug=False, enable_asserts=True
    )
    io = {
        "xT": nc.dram_tensor("xT", [F, N], F32R, kind="ExternalInput").ap(),
        "x": nc.dram_tensor("x", [N, F], F32R, kind="ExternalInput").ap(),
        "adj": nc.dram_tensor("adj", [N, N], DT_A, kind="ExternalInput").ap(),
        "w1": nc.dram_tensor("w1", [F, K], F32R, kind="ExternalInput").ap(),
        "w2": nc.dram_tensor("w2", [K, K], F32R, kind="ExternalInput").ap(),
        "b1": nc.dram_tensor("b1", [K], F32, kind="ExternalInput").ap(),
        "b2": nc.dram_tensor("b2", [K], F32, kind="ExternalInput").ap(),
        "mb": nc.dram_tensor("mb", [N], F32, kind="ExternalInput").ap(),
        "pfeat": nc.dram_tensor("pfeat", [K, F], F32, kind="ExternalOutput").ap(),
        "padj": nc.dram_tensor("padj", [K, K], F32, kind="ExternalOutput").ap(),
    }
    with tile.TileContext(nc) as tc, ExitStack() as ctx:
        _build_kernel(ctx, tc, io)
    nc.compile()
    _CACHE[key] = nc
    return nc


def make_in_maps(x, adj, mask, W1, b1, W2, b2):
    """Build the per-core input maps from the full (unsharded) inputs."""
    x = np.asarray(x, np.float32)
    adj = np.asarray(adj, np.float32)
    mask = np.asarray(mask, np.float32)
    w1 = np.ascontiguousarray(np.asarray(W1, np.float32))
    w2 = np.ascontiguousarray(np.asarray(W2, np.float32))
    b1v = np.ascontiguousarray(np.asarray(b1, np.float32))
    b2v = np.ascontiguousarray(np.asarray(b2, np.float32))
    in_maps = []
    for b in range(B):
        xb = np.ascontiguousarray(x[b])
        in_maps.append({
            "xT": np.ascontiguousarray(xb.T),
            "x": xb,
            "adj": np.ascontiguousarray(adj[b].astype(NP_A)),
            "w1": w1,
            "w2": w2,
            "b1": b1v,
            "b2": b2v,
            "mb": np.ascontiguousarray((-1e9 * (1.0 - mask[b])).astype(np.float32)),
        })
    return in_maps


def run(x, adj, mask, W1, b1, W2, b2, trace=False):
    nc = _get_nc()
    in_maps = make_in_maps(x, adj, mask, W1, b1, W2, b2)
    res = run_bass_kernel_spmd(nc, in_maps, core_ids=list(range(B)), trace=trace)
    pfeat = np.stack([res.results[b]["pfeat"] for b in range(B)]).astype(np.float32)
    padj = np.stack([res.results[b]["padj"] for b in range(B)]).astype(np.float32)
    pmask = np.ones((B, K), np.float32)
    return (pfeat, padj, pmask), res


def kernel(x, adj, mask, W1, b1, W2, b2):
    out, _ = run(x, adj, mask, W1, b1, W2, b2, trace=False)
    return out
